# revision 8
# baseline (speedup 1.0000x reference)
"""AttnGRU Trainium2 kernel: 8-way data-parallel, H-major dataflow.

Math per core (B_loc=32, T=128, H=1024):
  xr = x @ Wr_w.T + (Wr_b + Ur_b)      (precomputed, blocked over time)
  xn = x @ W_w.T  + W_b
  per step: rt = sigmoid(xr_t + h @ Ur_w.T)
            nt = tanh(xn_t + rt * (h @ U_w.T + U_b))
            h  = (1-gt)*nt + gt*h

Everything on-device lives H-major ("transposed"): tiles are
[128 partitions = H-chunk, free = (chunk, batch)] so elementwise ops use
all 128 partitions and the recurrent matmul output lands already in the
layout the next step consumes (no per-step transposes).

Changes vs the original baseline:
  - The per-step elementwise chain runs in two half-tiles (cols 0:128 /
    128:256 = h-chunks 0..3 / 4..7) so ACT and DVE pipeline against each
    other, and the next step's k=0..3 matmuls (which only read hbf cols
    0:128) start while the 4..7 half of the chain is still finishing.
    The matmul stream is ordered k-half-major to match.
  - h' = (1-g)*nt + g*h with the g*h term computed during the matmul
    burst (it only needs the previous h), so the critical chain per half
    is sigmoid -> mult -> add -> tanh -> mult -> add, all on ACT/DVE.
    The bf16 copy the matmuls consume (hbf) is produced directly by that
    last add; the fp32 state (hT) is a second add on GPSIMD, off-chain.
  - gtT input carries [g | 1-g] as [128, 512].
  - xT is DMAed in column slices so the first precompute block starts
    before the whole activation tensor has landed in SBUF.
  - Optional `reps`: repeats the whole computation (from h0) serially
    inside one NEFF — used by test.py to time per-execution cost with
    dispatch overhead amortized away.
"""

import numpy as np
import ml_dtypes

import concourse.bass as bass
import concourse.bacc as bacc
import concourse.mybir as mybir
from concourse import tile
from concourse.bass_utils import run_bass_kernel_spmd

B, T, H = 256, 128, 1024
NCORES = 8
BL = B // NCORES          # 32 batch rows per core
BT = BL * T               # 4096 (time-major: col = t*32 + b)
KC = H // 128             # 8 contraction chunks
MC = 2048 // 128          # 16 output chunks ([r | n] concat)
BLK = 8                   # scan steps per precompute block
NBLK = T // BLK           # 16
RING = 2 * BLK            # ring of per-step slots (512 cols each)

BF = mybir.dt.bfloat16
F32 = mybir.dt.float32
AF = mybir.ActivationFunctionType
OP = mybir.AluOpType

_CACHE = {}


def _build_bass(reps=1):
    nc = bacc.Bacc()
    xT = nc.declare_dram_parameter("xT", [H, BT], BF, isOutput=False)
    wpreT = nc.declare_dram_parameter("wpreT", [H, 2048], BF, isOutput=False)
    uuT = nc.declare_dram_parameter("uuT", [H, 2048], BF, isOutput=False)
    biasp = nc.declare_dram_parameter("biasp", [128, MC], F32, isOutput=False)
    ubT = nc.declare_dram_parameter("ubT", [128, 256], BF, isOutput=False)
    gtT = nc.declare_dram_parameter("gtT", [128, 512], F32, isOutput=False)
    h0T = nc.declare_dram_parameter("h0T", [128, 256], F32, isOutput=False)
    ident = nc.declare_dram_parameter("ident", [128, 128], BF, isOutput=False)
    out = nc.declare_dram_parameter("out", [128, 256], F32, isOutput=True)

    with tile.TileContext(nc) as tc:
        with (
            tc.tile_pool(name="w", bufs=1) as wp,
            tc.tile_pool(name="ew", bufs=4) as ew,
            tc.tile_pool(name="ps", bufs=1, space="PSUM") as psp,
            tc.tile_pool(name="pp", bufs=1, space="PSUM") as ppp,
        ):
            xT_sb = [wp.tile([128, BT], BF, tag=f"xT{k}", name=f"xT{k}") for k in range(KC)]
            uu_sb = [wp.tile([128, 2048], BF, tag=f"uu{k}", name=f"uu{k}") for k in range(KC)]
            wpre_sb = [wp.tile([128, 2048], BF, tag=f"wp{k}", name=f"wp{k}") for k in range(KC)]
            ring = wp.tile([128, RING * 512], BF, tag="ring")
            bias_sb = wp.tile([128, MC], F32, tag="bias")
            ub_sb = wp.tile([128, 256], BF, tag="ub")
            gt_sb = wp.tile([128, 512], F32, tag="gt")
            id_sb = wp.tile([128, 128], BF, tag="id")
            h0_sb = wp.tile([128, 256], F32, tag="h0")
            hT = wp.tile([128, 256], F32, tag="hT")
            hbf = wp.tile([128, 256], BF, tag="hbf")

            XSL = 4                       # xT DMA column slices
            for k in range(KC):
                nc.sync.dma_start(out=uu_sb[k][:, :], in_=uuT[k * 128:(k + 1) * 128, :])
                nc.sync.dma_start(out=wpre_sb[k][:, :], in_=wpreT[k * 128:(k + 1) * 128, :])
                for s in range(XSL):
                    cs = slice(s * (BT // XSL), (s + 1) * (BT // XSL))
                    nc.sync.dma_start(out=xT_sb[k][:, cs],
                                      in_=xT[k * 128:(k + 1) * 128, cs])
            nc.sync.dma_start(out=gt_sb[:, :], in_=gtT[:, :])
            nc.sync.dma_start(out=bias_sb[:, :], in_=biasp[:, :])
            nc.sync.dma_start(out=ub_sb[:, :], in_=ubT[:, :])
            nc.sync.dma_start(out=id_sb[:, :], in_=ident[:, :])
            nc.sync.dma_start(out=h0_sb[:, :], in_=h0T[:, :])

            ring3 = ring[:, :].rearrange("p (s c) -> p s c", c=512)

            NPRE = 2         # distinct precompute psum buffers

            def precompute_block(i, r):
                # xr/xn for steps i*BLK .. (i+1)*BLK, into ring slots (i%2)*BLK ..
                s0 = (i % 2) * BLK
                for m in range(MC):
                    slot = (i * MC + m) % NPRE
                    ps = ppp.tile([128, BLK * 32], F32, tag=f"pre{slot}",
                                  name=f"pre{r}_{i}_{m}", padded_shape=[128, 512])
                    for k in range(KC):
                        nc.tensor.matmul(
                            ps[:, :],
                            wpre_sb[k][:, m * 128:(m + 1) * 128],
                            xT_sb[k][:, i * BLK * 32:(i + 1) * BLK * 32],
                            start=(k == 0),
                            stop=(k == KC - 1),
                        )
                    dst = ring3[:, s0:s0 + BLK, m * 32:(m + 1) * 32]
                    src = ps[:, :].rearrange("p (s c) -> p s c", c=32)
                    nc.vector.tensor_scalar(dst, src, bias_sb[:, m:m + 1],
                                            None, OP.add)

            def scan_step(t, r):
                base = (t % RING) * 512
                slot = t % 3
                ps_r = psp.tile([128, 256], F32, tag=f"pr{slot}", name=f"psr{r}_{t}",
                                padded_shape=[128, 512])
                ps_n = psp.tile([128, 256], F32, tag=f"pn{slot}", name=f"psn{r}_{t}",
                                padded_shape=[128, 512])
                # Initialize PSUM via identity matmuls:
                # ps_r = xr_t (+r-biases, folded on host into the ring),
                # ps_n = U_b.
                nc.tensor.matmul(ps_r[:, :], id_sb[:, :], ring[:, base:base + 256],
                                 start=True, stop=False, skip_group_check=True)
                nc.tensor.matmul(ps_n[:, :], id_sb[:, :], ub_sb[:, :],
                                 start=True, stop=False, skip_group_check=True)
                # Four-phase stream. G0 = the m-chunks the first
                # elementwise half reads (ps_r m=0..3, ps_n m=8..11), G1 the
                # rest. Each group does its k=0..3 matmuls (which only need
                # hbf cols 0:128, the previous step's early product) and then
                # its k=4..7 matmuls, so G0 is fully accumulated after half
                # the stream and the elementwise chain overlaps the G1 half.
                G0 = (0, 1, 2, 3, 8, 9, 10, 11)
                G1 = (4, 5, 6, 7, 12, 13, 14, 15)
                for grp in (G0, G1):
                    for kh in range(2):
                        for m in grp:
                            half = ps_r if m < 8 else ps_n
                            col = (m % 8) * 32
                            for k in range(4 * kh, 4 * kh + 4):
                                nc.tensor.matmul(
                                    half[:, col:col + 32],
                                    uu_sb[k][:, m * 128:(m + 1) * 128],
                                    hbf[:, k * 32:(k + 1) * 32],
                                    start=False,
                                    stop=(k == KC - 1),
                                    skip_group_check=True,
                                )
                for hh in range(2):
                    hs = slice(128 * hh, 128 * (hh + 1))
                    g1 = slice(256 + 128 * hh, 256 + 128 * (hh + 1))
                    xn_b = slice(base + 256 + 128 * hh, base + 256 + 128 * (hh + 1))
                    # g*h term: depends only on the previous step's hT, so it
                    # runs during the matmul burst, off the critical chain.
                    p2 = ew.tile([128, 128], F32, tag=f"p2{hh}", name=f"p2{r}_{t}_{hh}")
                    nc.vector.tensor_tensor(p2[:, :], hT[:, hs], gt_sb[:, hs], OP.mult)
                    rt = ew.tile([128, 128], F32, tag=f"rt{hh}", name=f"rt{r}_{t}_{hh}")
                    nc.scalar.activation(rt[:, :], ps_r[:, hs], AF.Sigmoid)
                    n2 = ew.tile([128, 128], F32, tag=f"n2{hh}", name=f"n2{r}_{t}_{hh}")
                    nc.vector.tensor_tensor(n2[:, :], rt[:, :], ps_n[:, hs], OP.mult)
                    an = ew.tile([128, 128], F32, tag=f"an{hh}", name=f"an{r}_{t}_{hh}")
                    nc.vector.tensor_tensor(an[:, :], n2[:, :], ring[:, xn_b], OP.add)
                    nt = ew.tile([128, 128], F32, tag=f"nt{hh}", name=f"nt{r}_{t}_{hh}")
                    nc.scalar.activation(nt[:, :], an[:, :], AF.Tanh)
                    p1 = ew.tile([128, 128], F32, tag=f"p1{hh}", name=f"p1{r}_{t}_{hh}")
                    nc.vector.tensor_tensor(p1[:, :], nt[:, :], gt_sb[:, g1], OP.mult)
                    # critical: hbf feeds the next step's matmuls
                    nc.vector.tensor_tensor(hbf[:, hs], p1[:, :], p2[:, :], OP.add)
                    # fp32 state copy, off the critical chain
                    nc.gpsimd.tensor_tensor(hT[:, hs], p1[:, :], p2[:, :], OP.add)

            for r in range(reps):
                nc.vector.tensor_copy(hT[:, :], h0_sb[:, :])
                nc.vector.tensor_copy(hbf[:, :], h0_sb[:, :])
                precompute_block(0, r)
                precompute_block(1, r)
                for i in range(NBLK):
                    for u in range(BLK):
                        scan_step(i * BLK + u, r)
                    if i + 2 < NBLK:
                        precompute_block(i + 2, r)

            nc.sync.dma_start(out=out[:, :], in_=hT[:, :])

    nc.finalize()
    return nc


def _prep_inputs(x, h0, gt, Wr_w, Wr_b, Ur_w, Ur_b, W_w, W_b, U_w, U_b):
    bf = ml_dtypes.bfloat16
    wpreT = np.ascontiguousarray(
        np.concatenate([Wr_w.T, W_w.T], axis=1)).astype(bf)          # [H, 2048]
    uuT = np.ascontiguousarray(
        np.concatenate([Ur_w.T, U_w.T], axis=1)).astype(bf)          # [H, 2048]
    # biasp[:, m] = per-partition bias for precompute chunk m, added during
    # the psum->ring copy: r-chunks get Wr_b+Ur_b, n-chunks get W_b.
    # ubT = U_b broadcast (folded into ps_n by an identity matmul).
    def hmajor_bcast(v):
        return np.ascontiguousarray(
            np.broadcast_to(v.reshape(8, 128).T[:, :, None],
                            (128, 8, 32)).reshape(128, 256))
    bias_cat = np.concatenate([(Wr_b + Ur_b), W_b]).astype(np.float32)
    biasp = np.ascontiguousarray(bias_cat.reshape(MC, 128).T)
    ubT = hmajor_bcast(U_b.astype(np.float32)).astype(bf)
    ident = np.eye(128, dtype=bf)

    in_maps = []
    for c in range(NCORES):
        sl = slice(c * BL, (c + 1) * BL)
        x_loc = x[sl]                                  # [32, 128, 1024]
        xT = np.ascontiguousarray(
            x_loc.transpose(2, 1, 0).reshape(H, BT)).astype(bf)
        h0T = np.ascontiguousarray(
            h0[sl].reshape(BL, 8, 128).transpose(2, 1, 0).reshape(128, 256)
        ).astype(np.float32)
        g_b = np.ascontiguousarray(
            np.broadcast_to(gt[sl].reshape(BL)[None, None, :],
                            (128, 8, 32)).reshape(128, 256)).astype(np.float32)
        gtT = np.ascontiguousarray(np.concatenate([g_b, 1.0 - g_b], axis=1))
        in_maps.append({
            "xT": xT, "wpreT": wpreT, "uuT": uuT, "biasp": biasp,
            "ubT": ubT, "gtT": gtT, "h0T": h0T, "ident": ident,
        })
    return in_maps


def kernel(x, h0, gt, Wr_w, Wr_b, Ur_w, Ur_b, Wz_w, Wz_b, Uz_w, Uz_b,
           W_w, W_b, U_w, U_b, _trace=False, _tmpdir=None):
    x = np.asarray(x, np.float32)
    h0 = np.asarray(h0, np.float32)
    gt = np.asarray(gt, np.float32)
    in_maps = _prep_inputs(x, h0, gt,
                           np.asarray(Wr_w, np.float32), np.asarray(Wr_b, np.float32),
                           np.asarray(Ur_w, np.float32), np.asarray(Ur_b, np.float32),
                           np.asarray(W_w, np.float32), np.asarray(W_b, np.float32),
                           np.asarray(U_w, np.float32), np.asarray(U_b, np.float32))
    if "nc" not in _CACHE:
        _CACHE["nc"] = _build_bass()
    res = run_bass_kernel_spmd(_CACHE["nc"], in_maps, core_ids=list(range(NCORES)),
                               trace=_trace, tmpdir=_tmpdir)
    outs = []
    for c in range(NCORES):
        o = np.asarray(res.results[c]["out"], np.float32)       # [128, 256]
        outs.append(o.reshape(128, 8, BL).transpose(2, 1, 0).reshape(BL, H))
    full = np.concatenate(outs, axis=0)                          # [256, 1024]
    if _trace:
        return full, res
    return full


# revision 9
# speedup vs baseline: 1.3887x; 1.3887x over previous
"""AttnGRU Trainium2 kernel: 8-way data-parallel, H-major dataflow.

Math per core (B_loc=32, T=128, H=1024):
  xr = x @ Wr_w.T + (Wr_b + Ur_b)      (precomputed, blocked over time)
  xn = x @ W_w.T  + W_b
  per step: rt = sigmoid(xr_t + h @ Ur_w.T)
            nt = tanh(xn_t + rt * (h @ U_w.T + U_b))
            h  = (1-gt)*nt + gt*h

Everything on-device lives H-major ("transposed"): tiles are
[128 partitions = H-chunk, free = (chunk, batch)] so elementwise ops use
all 128 partitions and the recurrent matmul output lands already in the
layout the next step consumes (no per-step transposes).

Changes vs the original baseline:
  - The per-step elementwise chain runs in two half-tiles (cols 0:128 /
    128:256 = h-chunks 0..3 / 4..7) so ACT and DVE pipeline against each
    other, and the next step's k=0..3 matmuls (which only read hbf cols
    0:128) start while the 4..7 half of the chain is still finishing.
    The matmul stream is ordered k-half-major to match.
  - h' = (1-g)*nt + g*h with the g*h term computed during the matmul
    burst (it only needs the previous h), so the critical chain per half
    is sigmoid -> mult -> add -> tanh -> mult -> add, all on ACT/DVE.
    The bf16 copy the matmuls consume (hbf) is produced directly by that
    last add; the fp32 state (hT) is a second add on GPSIMD, off-chain.
  - gtT input carries [g | 1-g] as [128, 512].
  - xT is DMAed in column slices so the first precompute block starts
    before the whole activation tensor has landed in SBUF.
  - Optional `reps`: repeats the whole computation (from h0) serially
    inside one NEFF — used by test.py to time per-execution cost with
    dispatch overhead amortized away.
"""

import numpy as np
import ml_dtypes

import concourse.bass as bass
import concourse.bacc as bacc
import concourse.mybir as mybir
from concourse import tile
from concourse.bass_utils import run_bass_kernel_spmd

B, T, H = 256, 128, 1024
NCORES = 8
BL = B // NCORES          # 32 batch rows per core
BT = BL * T               # 4096 (time-major: col = t*32 + b)
KC = H // 128             # 8 contraction chunks
MC = 2048 // 128          # 16 output chunks ([r | n] concat)
BLK = 8                   # scan steps per precompute block
NBLK = T // BLK           # 16
RING = 2 * BLK            # ring of per-step slots (512 cols each)

BF = mybir.dt.bfloat16
F32 = mybir.dt.float32
AF = mybir.ActivationFunctionType
OP = mybir.AluOpType

_CACHE = {}


def _build_bass(reps=1):
    nc = bacc.Bacc()
    xT = nc.declare_dram_parameter("xT", [H, BT], BF, isOutput=False)
    wpreT = nc.declare_dram_parameter("wpreT", [H, 2048], BF, isOutput=False)
    uuT = nc.declare_dram_parameter("uuT", [H, 2048], BF, isOutput=False)
    biasp = nc.declare_dram_parameter("biasp", [128, MC], F32, isOutput=False)
    ubT = nc.declare_dram_parameter("ubT", [128, 256], BF, isOutput=False)
    gtT = nc.declare_dram_parameter("gtT", [128, 512], F32, isOutput=False)
    h0T = nc.declare_dram_parameter("h0T", [128, 256], F32, isOutput=False)
    ident = nc.declare_dram_parameter("ident", [128, 128], BF, isOutput=False)
    out = nc.declare_dram_parameter("out", [128, 256], F32, isOutput=True)

    with tile.TileContext(nc) as tc:
        with (
            tc.tile_pool(name="w", bufs=1) as wp,
            tc.tile_pool(name="ew", bufs=3) as ew,
            tc.tile_pool(name="ps", bufs=1, space="PSUM") as psp,
            tc.tile_pool(name="pp", bufs=1, space="PSUM") as ppp,
        ):
            xT_sb = [wp.tile([128, BT], BF, tag=f"xT{k}", name=f"xT{k}") for k in range(KC)]
            uu_sb = [wp.tile([128, 2048], BF, tag=f"uu{k}", name=f"uu{k}") for k in range(KC)]
            wpre_sb = [wp.tile([128, 2048], BF, tag=f"wp{k}", name=f"wp{k}") for k in range(KC)]
            ring = wp.tile([128, RING * 512], BF, tag="ring")
            bias_sb = wp.tile([128, MC], F32, tag="bias")
            ub_sb = wp.tile([128, 256], BF, tag="ub")
            gt_sb = wp.tile([128, 512], F32, tag="gt")
            id_sb = wp.tile([128, 128], BF, tag="id")
            h0_sb = wp.tile([128, 256], F32, tag="h0")
            hT = wp.tile([128, 256], F32, tag="hT")
            hbf = wp.tile([128, 256], BF, tag="hbf")

            XSL = 4                       # xT DMA column slices
            for k in range(KC):
                nc.sync.dma_start(out=uu_sb[k][:, :], in_=uuT[k * 128:(k + 1) * 128, :])
                nc.sync.dma_start(out=wpre_sb[k][:, :], in_=wpreT[k * 128:(k + 1) * 128, :])
                for s in range(XSL):
                    cs = slice(s * (BT // XSL), (s + 1) * (BT // XSL))
                    nc.sync.dma_start(out=xT_sb[k][:, cs],
                                      in_=xT[k * 128:(k + 1) * 128, cs])
            nc.sync.dma_start(out=gt_sb[:, :], in_=gtT[:, :])
            nc.sync.dma_start(out=bias_sb[:, :], in_=biasp[:, :])
            nc.sync.dma_start(out=ub_sb[:, :], in_=ubT[:, :])
            nc.sync.dma_start(out=id_sb[:, :], in_=ident[:, :])
            nc.sync.dma_start(out=h0_sb[:, :], in_=h0T[:, :])

            ring3 = ring[:, :].rearrange("p (s c) -> p s c", c=512)

            NPRE = 4         # distinct precompute psum buffers

            def precompute_block(i, r):
                # xr/xn for steps i*BLK .. (i+1)*BLK, into ring slots (i%2)*BLK ..
                s0 = (i % 2) * BLK
                for m in range(MC):
                    slot = (i * MC + m) % NPRE
                    ps = ppp.tile([128, BLK * 32], F32, tag=f"pre{slot}",
                                  name=f"pre{r}_{i}_{m}", padded_shape=[128, 512])
                    for k in range(KC):
                        nc.tensor.matmul(
                            ps[:, :],
                            wpre_sb[k][:, m * 128:(m + 1) * 128],
                            xT_sb[k][:, i * BLK * 32:(i + 1) * BLK * 32],
                            start=(k == 0),
                            stop=(k == KC - 1),
                        )
                    dst = ring3[:, s0:s0 + BLK, m * 32:(m + 1) * 32]
                    src = ps[:, :].rearrange("p (s c) -> p s c", c=32)
                    nc.vector.tensor_scalar(dst, src, bias_sb[:, m:m + 1],
                                            None, OP.add)

            def scan_step(t, r):
                base = (t % RING) * 512
                slot = t % 2
                ps_r = psp.tile([128, 256], F32, tag=f"pr{slot}", name=f"psr{r}_{t}",
                                padded_shape=[128, 512])
                ps_n = psp.tile([128, 256], F32, tag=f"pn{slot}", name=f"psn{r}_{t}",
                                padded_shape=[128, 512])
                # Initialize PSUM via identity matmuls:
                # ps_r = xr_t (+r-biases, folded on host into the ring),
                # ps_n = U_b.
                nc.tensor.matmul(ps_r[:, :], id_sb[:, :], ring[:, base:base + 256],
                                 start=True, stop=False, skip_group_check=True)
                nc.tensor.matmul(ps_n[:, :], id_sb[:, :], ub_sb[:, :],
                                 start=True, stop=False, skip_group_check=True)
                # k-half-major: the kh=0 matmuls only read hbf cols 0:128,
                # which the previous step's first elementwise half produced.
                for kh in range(2):
                    for m in range(MC):
                        half = ps_r if m < 8 else ps_n
                        col = (m % 8) * 32
                        for k in range(4 * kh, 4 * kh + 4):
                            nc.tensor.matmul(
                                half[:, col:col + 32],
                                uu_sb[k][:, m * 128:(m + 1) * 128],
                                hbf[:, k * 32:(k + 1) * 32],
                                start=False,
                                stop=(k == KC - 1),
                                skip_group_check=True,
                            )
                for hh in range(2):
                    hs = slice(128 * hh, 128 * (hh + 1))
                    g1 = slice(256 + 128 * hh, 256 + 128 * (hh + 1))
                    xn_b = slice(base + 256 + 128 * hh, base + 256 + 128 * (hh + 1))
                    # g*h term: depends only on the previous step's hT, so it
                    # runs during the matmul burst, off the critical chain.
                    p2 = ew.tile([128, 128], F32, tag=f"p2{hh}", name=f"p2{r}_{t}_{hh}")
                    nc.vector.tensor_tensor(p2[:, :], hT[:, hs], gt_sb[:, hs], OP.mult)
                    rt = ew.tile([128, 128], F32, tag=f"rt{hh}", name=f"rt{r}_{t}_{hh}")
                    nc.scalar.activation(rt[:, :], ps_r[:, hs], AF.Sigmoid)
                    n2 = ew.tile([128, 128], F32, tag=f"n2{hh}", name=f"n2{r}_{t}_{hh}")
                    nc.vector.tensor_tensor(n2[:, :], rt[:, :], ps_n[:, hs], OP.mult)
                    an = ew.tile([128, 128], F32, tag=f"an{hh}", name=f"an{r}_{t}_{hh}")
                    nc.vector.tensor_tensor(an[:, :], n2[:, :], ring[:, xn_b], OP.add)
                    nt = ew.tile([128, 128], F32, tag=f"nt{hh}", name=f"nt{r}_{t}_{hh}")
                    nc.scalar.activation(nt[:, :], an[:, :], AF.Tanh)
                    p1 = ew.tile([128, 128], F32, tag=f"p1{hh}", name=f"p1{r}_{t}_{hh}")
                    nc.vector.tensor_tensor(p1[:, :], nt[:, :], gt_sb[:, g1], OP.mult)
                    # critical: hbf feeds the next step's matmuls
                    nc.vector.tensor_tensor(hbf[:, hs], p1[:, :], p2[:, :], OP.add)
                    # fp32 state copy, off the critical chain
                    nc.gpsimd.tensor_tensor(hT[:, hs], p1[:, :], p2[:, :], OP.add)

            for r in range(reps):
                nc.vector.tensor_copy(hT[:, :], h0_sb[:, :])
                nc.vector.tensor_copy(hbf[:, :], h0_sb[:, :])
                precompute_block(0, r)
                precompute_block(1, r)
                for i in range(NBLK):
                    for u in range(BLK):
                        scan_step(i * BLK + u, r)
                    if i + 2 < NBLK:
                        precompute_block(i + 2, r)

            nc.sync.dma_start(out=out[:, :], in_=hT[:, :])

    nc.finalize()
    return nc


def _prep_inputs(x, h0, gt, Wr_w, Wr_b, Ur_w, Ur_b, W_w, W_b, U_w, U_b):
    bf = ml_dtypes.bfloat16
    wpreT = np.ascontiguousarray(
        np.concatenate([Wr_w.T, W_w.T], axis=1)).astype(bf)          # [H, 2048]
    uuT = np.ascontiguousarray(
        np.concatenate([Ur_w.T, U_w.T], axis=1)).astype(bf)          # [H, 2048]
    # biasp[:, m] = per-partition bias for precompute chunk m, added during
    # the psum->ring copy: r-chunks get Wr_b+Ur_b, n-chunks get W_b.
    # ubT = U_b broadcast (folded into ps_n by an identity matmul).
    def hmajor_bcast(v):
        return np.ascontiguousarray(
            np.broadcast_to(v.reshape(8, 128).T[:, :, None],
                            (128, 8, 32)).reshape(128, 256))
    bias_cat = np.concatenate([(Wr_b + Ur_b), W_b]).astype(np.float32)
    biasp = np.ascontiguousarray(bias_cat.reshape(MC, 128).T)
    ubT = hmajor_bcast(U_b.astype(np.float32)).astype(bf)
    ident = np.eye(128, dtype=bf)

    in_maps = []
    for c in range(NCORES):
        sl = slice(c * BL, (c + 1) * BL)
        x_loc = x[sl]                                  # [32, 128, 1024]
        xT = np.ascontiguousarray(
            x_loc.transpose(2, 1, 0).reshape(H, BT)).astype(bf)
        h0T = np.ascontiguousarray(
            h0[sl].reshape(BL, 8, 128).transpose(2, 1, 0).reshape(128, 256)
        ).astype(np.float32)
        g_b = np.ascontiguousarray(
            np.broadcast_to(gt[sl].reshape(BL)[None, None, :],
                            (128, 8, 32)).reshape(128, 256)).astype(np.float32)
        gtT = np.ascontiguousarray(np.concatenate([g_b, 1.0 - g_b], axis=1))
        in_maps.append({
            "xT": xT, "wpreT": wpreT, "uuT": uuT, "biasp": biasp,
            "ubT": ubT, "gtT": gtT, "h0T": h0T, "ident": ident,
        })
    return in_maps


def kernel(x, h0, gt, Wr_w, Wr_b, Ur_w, Ur_b, Wz_w, Wz_b, Uz_w, Uz_b,
           W_w, W_b, U_w, U_b, _trace=False, _tmpdir=None):
    x = np.asarray(x, np.float32)
    h0 = np.asarray(h0, np.float32)
    gt = np.asarray(gt, np.float32)
    in_maps = _prep_inputs(x, h0, gt,
                           np.asarray(Wr_w, np.float32), np.asarray(Wr_b, np.float32),
                           np.asarray(Ur_w, np.float32), np.asarray(Ur_b, np.float32),
                           np.asarray(W_w, np.float32), np.asarray(W_b, np.float32),
                           np.asarray(U_w, np.float32), np.asarray(U_b, np.float32))
    if "nc" not in _CACHE:
        _CACHE["nc"] = _build_bass()
    res = run_bass_kernel_spmd(_CACHE["nc"], in_maps, core_ids=list(range(NCORES)),
                               trace=_trace, tmpdir=_tmpdir)
    outs = []
    for c in range(NCORES):
        o = np.asarray(res.results[c]["out"], np.float32)       # [128, 256]
        outs.append(o.reshape(128, 8, BL).transpose(2, 1, 0).reshape(BL, H))
    full = np.concatenate(outs, axis=0)                          # [256, 1024]
    if _trace:
        return full, res
    return full


# revision 12
# speedup vs baseline: 1.4087x; 1.0144x over previous
"""AttnGRU Trainium2 kernel: 8-way data-parallel, H-major dataflow.

Math per core (B_loc=32, T=128, H=1024):
  xr = x @ Wr_w.T + (Wr_b + Ur_b)      (precomputed, blocked over time)
  xn = x @ W_w.T  + W_b
  per step: rt = sigmoid(xr_t + h @ Ur_w.T)
            nt = tanh(xn_t + rt * (h @ U_w.T + U_b))
            h  = (1-gt)*nt + gt*h

Everything on-device lives H-major ("transposed"): tiles are
[128 partitions = H-chunk, free = (chunk, batch)] so elementwise ops use
all 128 partitions and the recurrent matmul output lands already in the
layout the next step consumes (no per-step transposes).

Changes vs the original baseline:
  - The per-step elementwise chain runs in two half-tiles (cols 0:128 /
    128:256 = h-chunks 0..3 / 4..7) so ACT and DVE pipeline against each
    other, and the next step's k=0..3 matmuls (which only read hbf cols
    0:128) start while the 4..7 half of the chain is still finishing.
    The matmul stream is ordered k-half-major to match.
  - h' = (1-g)*nt + g*h with the g*h term computed during the matmul
    burst (it only needs the previous h), so the critical chain per half
    is sigmoid -> mult -> add -> tanh -> mult -> add, all on ACT/DVE.
    The bf16 copy the matmuls consume (hbf) is produced directly by that
    last add; the fp32 state (hT) is a second add on GPSIMD, off-chain.
  - gtT input carries [g | 1-g] as [128, 512].
  - xT is DMAed in column slices so the first precompute block starts
    before the whole activation tensor has landed in SBUF.
  - Optional `reps`: repeats the whole computation (from h0) serially
    inside one NEFF — used by test.py to time per-execution cost with
    dispatch overhead amortized away.
"""

import numpy as np
import ml_dtypes

import concourse.bass as bass
import concourse.bacc as bacc
import concourse.mybir as mybir
from concourse import tile
from concourse.bass_utils import run_bass_kernel_spmd

B, T, H = 256, 128, 1024
NCORES = 8
BL = B // NCORES          # 32 batch rows per core
BT = BL * T               # 4096 (time-major: col = t*32 + b)
KC = H // 128             # 8 contraction chunks
MC = 2048 // 128          # 16 output chunks ([r | n] concat)
BLK = 8                   # scan steps per precompute block
NBLK = T // BLK           # 16
RING = 2 * BLK            # ring of per-step slots (512 cols each)

BF = mybir.dt.bfloat16
F32 = mybir.dt.float32
AF = mybir.ActivationFunctionType
OP = mybir.AluOpType

_CACHE = {}


def _build_bass(reps=1):
    nc = bacc.Bacc()
    xT = nc.declare_dram_parameter("xT", [H, BT], BF, isOutput=False)
    wpreT = nc.declare_dram_parameter("wpreT", [H, 2048], BF, isOutput=False)
    uuT = nc.declare_dram_parameter("uuT", [H, 2048], BF, isOutput=False)
    biasp = nc.declare_dram_parameter("biasp", [128, MC], F32, isOutput=False)
    ubT = nc.declare_dram_parameter("ubT", [128, 256], BF, isOutput=False)
    gtT = nc.declare_dram_parameter("gtT", [128, 512], F32, isOutput=False)
    h0T = nc.declare_dram_parameter("h0T", [128, 256], F32, isOutput=False)
    ident = nc.declare_dram_parameter("ident", [128, 128], BF, isOutput=False)
    out = nc.declare_dram_parameter("out", [128, 256], F32, isOutput=True)

    with tile.TileContext(nc) as tc:
        with (
            tc.tile_pool(name="w", bufs=1) as wp,
            tc.tile_pool(name="ew", bufs=3) as ew,
            tc.tile_pool(name="ps", bufs=1, space="PSUM") as psp,
            tc.tile_pool(name="pp", bufs=1, space="PSUM") as ppp,
        ):
            xT_sb = [wp.tile([128, BT], BF, tag=f"xT{k}", name=f"xT{k}") for k in range(KC)]
            uu_sb = [wp.tile([128, 2048], BF, tag=f"uu{k}", name=f"uu{k}") for k in range(KC)]
            wpre_sb = [wp.tile([128, 2048], BF, tag=f"wp{k}", name=f"wp{k}") for k in range(KC)]
            ring = wp.tile([128, RING * 512], BF, tag="ring")
            bias_sb = wp.tile([128, MC], F32, tag="bias")
            ub_sb = wp.tile([128, 256], BF, tag="ub")
            gt_sb = wp.tile([128, 512], F32, tag="gt")
            id_sb = wp.tile([128, 128], BF, tag="id")
            h0_sb = wp.tile([128, 256], F32, tag="h0")
            hT = wp.tile([128, 256], F32, tag="hT")
            hbf = wp.tile([128, 256], BF, tag="hbf")

            XSL = 4                       # xT DMA column slices
            for k in range(KC):
                nc.sync.dma_start(out=uu_sb[k][:, :], in_=uuT[k * 128:(k + 1) * 128, :])
                nc.sync.dma_start(out=wpre_sb[k][:, :], in_=wpreT[k * 128:(k + 1) * 128, :])
                for s in range(XSL):
                    cs = slice(s * (BT // XSL), (s + 1) * (BT // XSL))
                    nc.sync.dma_start(out=xT_sb[k][:, cs],
                                      in_=xT[k * 128:(k + 1) * 128, cs])
            nc.sync.dma_start(out=gt_sb[:, :], in_=gtT[:, :])
            nc.sync.dma_start(out=bias_sb[:, :], in_=biasp[:, :])
            nc.sync.dma_start(out=ub_sb[:, :], in_=ubT[:, :])
            nc.sync.dma_start(out=id_sb[:, :], in_=ident[:, :])
            nc.sync.dma_start(out=h0_sb[:, :], in_=h0T[:, :])

            ring3 = ring[:, :].rearrange("p (s c) -> p s c", c=512)

            NPRE = 4         # distinct precompute psum buffers

            def precompute_block(i, r):
                # xr/xn for steps i*BLK .. (i+1)*BLK, into ring slots (i%2)*BLK ..
                s0 = (i % 2) * BLK
                for m in range(MC):
                    slot = (i * MC + m) % NPRE
                    ps = ppp.tile([128, BLK * 32], F32, tag=f"pre{slot}",
                                  name=f"pre{r}_{i}_{m}", padded_shape=[128, 512])
                    for k in range(KC):
                        nc.tensor.matmul(
                            ps[:, :],
                            wpre_sb[k][:, m * 128:(m + 1) * 128],
                            xT_sb[k][:, i * BLK * 32:(i + 1) * BLK * 32],
                            start=(k == 0),
                            stop=(k == KC - 1),
                        )
                    dst = ring3[:, s0:s0 + BLK, m * 32:(m + 1) * 32]
                    src = ps[:, :].rearrange("p (s c) -> p s c", c=32)
                    nc.vector.tensor_scalar(dst, src, bias_sb[:, m:m + 1],
                                            None, OP.add)

            def scan_step(t, r):
                base = (t % RING) * 512
                slot = t % 2
                ps_r = psp.tile([128, 256], F32, tag=f"pr{slot}", name=f"psr{r}_{t}",
                                padded_shape=[128, 512])
                ps_n = psp.tile([128, 256], F32, tag=f"pn{slot}", name=f"psn{r}_{t}",
                                padded_shape=[128, 512])
                # No identity-matmul PSUM init: the first matmul into each
                # bank (m==0 / m==8 at kh==0,k==0) carries start=True, which
                # clears the bank's has_written bits; every later start=False
                # matmul overwrites-where-unset / accumulates-where-set per
                # element, so each 32-col region self-initializes. The xr and
                # U_b addends are applied by full-width DVE adds below.
                # k-half-major: the kh=0 matmuls only read hbf cols 0:128,
                # which the previous step's first elementwise half produced.
                for kh in range(2):
                    for m in range(MC):
                        half = ps_r if m < 8 else ps_n
                        col = (m % 8) * 32
                        for k in range(4 * kh, 4 * kh + 4):
                            nc.tensor.matmul(
                                half[:, col:col + 32],
                                uu_sb[k][:, m * 128:(m + 1) * 128],
                                hbf[:, k * 32:(k + 1) * 32],
                                start=(kh == 0 and k == 0 and m % 8 == 0),
                                stop=(k == KC - 1),
                                skip_group_check=True,
                            )
                for hh in range(2):
                    hs = slice(128 * hh, 128 * (hh + 1))
                    g1 = slice(256 + 128 * hh, 256 + 128 * (hh + 1))
                    xn_b = slice(base + 256 + 128 * hh, base + 256 + 128 * (hh + 1))
                    # g*h term: depends only on the previous step's hT, so it
                    # runs during the matmul burst, off the critical chain.
                    p2 = ew.tile([128, 128], F32, tag=f"p2{hh}", name=f"p2{r}_{t}_{hh}")
                    nc.vector.tensor_tensor(p2[:, :], hT[:, hs], gt_sb[:, hs], OP.mult)
                    rs = ew.tile([128, 128], F32, tag=f"rs{hh}", name=f"rs{r}_{t}_{hh}")
                    nc.vector.tensor_tensor(rs[:, :], ps_r[:, hs],
                                            ring[:, base + 128 * hh:base + 128 * (hh + 1)],
                                            OP.add)
                    rt = ew.tile([128, 128], F32, tag=f"rt{hh}", name=f"rt{r}_{t}_{hh}")
                    nc.scalar.activation(rt[:, :], rs[:, :], AF.Sigmoid)
                    sn = ew.tile([128, 128], F32, tag=f"sn{hh}", name=f"sn{r}_{t}_{hh}")
                    nc.vector.tensor_tensor(sn[:, :], ps_n[:, hs], ub_sb[:, hs], OP.add)
                    n2 = ew.tile([128, 128], F32, tag=f"n2{hh}", name=f"n2{r}_{t}_{hh}")
                    nc.vector.tensor_tensor(n2[:, :], rt[:, :], sn[:, :], OP.mult)
                    an = ew.tile([128, 128], F32, tag=f"an{hh}", name=f"an{r}_{t}_{hh}")
                    nc.vector.tensor_tensor(an[:, :], n2[:, :], ring[:, xn_b], OP.add)
                    nt = ew.tile([128, 128], F32, tag=f"nt{hh}", name=f"nt{r}_{t}_{hh}")
                    nc.scalar.activation(nt[:, :], an[:, :], AF.Tanh)
                    p1 = ew.tile([128, 128], F32, tag=f"p1{hh}", name=f"p1{r}_{t}_{hh}")
                    nc.vector.tensor_tensor(p1[:, :], nt[:, :], gt_sb[:, g1], OP.mult)
                    # critical: hbf feeds the next step's matmuls
                    nc.vector.tensor_tensor(hbf[:, hs], p1[:, :], p2[:, :], OP.add)
                    # fp32 state copy, off the critical chain
                    nc.gpsimd.tensor_tensor(hT[:, hs], p1[:, :], p2[:, :], OP.add)

            for r in range(reps):
                nc.vector.tensor_copy(hT[:, :], h0_sb[:, :])
                nc.vector.tensor_copy(hbf[:, :], h0_sb[:, :])
                precompute_block(0, r)
                precompute_block(1, r)
                for i in range(NBLK):
                    for u in range(BLK):
                        scan_step(i * BLK + u, r)
                    if i + 2 < NBLK:
                        precompute_block(i + 2, r)

            nc.sync.dma_start(out=out[:, :], in_=hT[:, :])

    nc.finalize()
    return nc


def _prep_inputs(x, h0, gt, Wr_w, Wr_b, Ur_w, Ur_b, W_w, W_b, U_w, U_b):
    bf = ml_dtypes.bfloat16
    wpreT = np.ascontiguousarray(
        np.concatenate([Wr_w.T, W_w.T], axis=1)).astype(bf)          # [H, 2048]
    uuT = np.ascontiguousarray(
        np.concatenate([Ur_w.T, U_w.T], axis=1)).astype(bf)          # [H, 2048]
    # biasp[:, m] = per-partition bias for precompute chunk m, added during
    # the psum->ring copy: r-chunks get Wr_b+Ur_b, n-chunks get W_b.
    # ubT = U_b broadcast (folded into ps_n by an identity matmul).
    def hmajor_bcast(v):
        return np.ascontiguousarray(
            np.broadcast_to(v.reshape(8, 128).T[:, :, None],
                            (128, 8, 32)).reshape(128, 256))
    bias_cat = np.concatenate([(Wr_b + Ur_b), W_b]).astype(np.float32)
    biasp = np.ascontiguousarray(bias_cat.reshape(MC, 128).T)
    ubT = hmajor_bcast(U_b.astype(np.float32)).astype(bf)
    ident = np.eye(128, dtype=bf)

    in_maps = []
    for c in range(NCORES):
        sl = slice(c * BL, (c + 1) * BL)
        x_loc = x[sl]                                  # [32, 128, 1024]
        xT = np.ascontiguousarray(
            x_loc.transpose(2, 1, 0).reshape(H, BT)).astype(bf)
        h0T = np.ascontiguousarray(
            h0[sl].reshape(BL, 8, 128).transpose(2, 1, 0).reshape(128, 256)
        ).astype(np.float32)
        g_b = np.ascontiguousarray(
            np.broadcast_to(gt[sl].reshape(BL)[None, None, :],
                            (128, 8, 32)).reshape(128, 256)).astype(np.float32)
        gtT = np.ascontiguousarray(np.concatenate([g_b, 1.0 - g_b], axis=1))
        in_maps.append({
            "xT": xT, "wpreT": wpreT, "uuT": uuT, "biasp": biasp,
            "ubT": ubT, "gtT": gtT, "h0T": h0T, "ident": ident,
        })
    return in_maps


def kernel(x, h0, gt, Wr_w, Wr_b, Ur_w, Ur_b, Wz_w, Wz_b, Uz_w, Uz_b,
           W_w, W_b, U_w, U_b, _trace=False, _tmpdir=None):
    x = np.asarray(x, np.float32)
    h0 = np.asarray(h0, np.float32)
    gt = np.asarray(gt, np.float32)
    in_maps = _prep_inputs(x, h0, gt,
                           np.asarray(Wr_w, np.float32), np.asarray(Wr_b, np.float32),
                           np.asarray(Ur_w, np.float32), np.asarray(Ur_b, np.float32),
                           np.asarray(W_w, np.float32), np.asarray(W_b, np.float32),
                           np.asarray(U_w, np.float32), np.asarray(U_b, np.float32))
    if "nc" not in _CACHE:
        _CACHE["nc"] = _build_bass()
    res = run_bass_kernel_spmd(_CACHE["nc"], in_maps, core_ids=list(range(NCORES)),
                               trace=_trace, tmpdir=_tmpdir)
    outs = []
    for c in range(NCORES):
        o = np.asarray(res.results[c]["out"], np.float32)       # [128, 256]
        outs.append(o.reshape(128, 8, BL).transpose(2, 1, 0).reshape(BL, H))
    full = np.concatenate(outs, axis=0)                          # [256, 1024]
    if _trace:
        return full, res
    return full


# revision 13
# speedup vs baseline: 2.8025x; 1.9894x over previous
"""AttnGRU Trainium2 kernel: 8-way data-parallel, H-major dataflow.

Math per core (B_loc=32, T=128, H=1024):
  xr = x @ Wr_w.T + (Wr_b + Ur_b)      (precomputed, blocked over time)
  xn = x @ W_w.T  + W_b
  per step: rt = sigmoid(xr_t + h @ Ur_w.T)
            nt = tanh(xn_t + rt * (h @ U_w.T + U_b))
            h  = (1-gt)*nt + gt*h

Everything on-device lives H-major ("transposed"): tiles are
[128 partitions = H-chunk, free = (chunk, batch)] so elementwise ops use
all 128 partitions and the recurrent matmul output lands already in the
layout the next step consumes (no per-step transposes).

Changes vs the original baseline:
  - The per-step elementwise chain runs in two half-tiles (cols 0:128 /
    128:256 = h-chunks 0..3 / 4..7) so ACT and DVE pipeline against each
    other, and the next step's k=0..3 matmuls (which only read hbf cols
    0:128) start while the 4..7 half of the chain is still finishing.
    The matmul stream is ordered k-half-major to match.
  - h' = (1-g)*nt + g*h with the g*h term computed during the matmul
    burst (it only needs the previous h), so the critical chain per half
    is sigmoid -> mult -> add -> tanh -> mult -> add, all on ACT/DVE.
    The bf16 copy the matmuls consume (hbf) is produced directly by that
    last add; the fp32 state (hT) is a second add on GPSIMD, off-chain.
  - gtT input carries [g | 1-g] as [128, 512].
  - xT is DMAed in column slices so the first precompute block starts
    before the whole activation tensor has landed in SBUF.
  - Optional `reps`: repeats the whole computation (from h0) serially
    inside one NEFF — used by test.py to time per-execution cost with
    dispatch overhead amortized away.
"""

import numpy as np
import ml_dtypes

import concourse.bass as bass
import concourse.bacc as bacc
import concourse.mybir as mybir
from concourse import tile
from concourse.bass_utils import run_bass_kernel_spmd

B, T, H = 256, 128, 1024
NCORES = 8
BL = B // NCORES          # 32 batch rows per core
BT = BL * T               # 4096 (time-major: col = t*32 + b)
KC = H // 128             # 8 contraction chunks
MC = 2048 // 128          # 16 output chunks ([r | n] concat)
BLK = 8                   # scan steps per precompute block
NBLK = T // BLK           # 16
RING = 2 * BLK            # ring of per-step slots (512 cols each)

BF = mybir.dt.bfloat16
F32 = mybir.dt.float32
AF = mybir.ActivationFunctionType
OP = mybir.AluOpType

_CACHE = {}


def _build_bass(reps=1):
    nc = bacc.Bacc()
    xT = nc.declare_dram_parameter("xT", [H, BT], BF, isOutput=False)
    wpreT = nc.declare_dram_parameter("wpreT", [H, 2048], BF, isOutput=False)
    uuT = nc.declare_dram_parameter("uuT", [H, 2048], BF, isOutput=False)
    biasp = nc.declare_dram_parameter("biasp", [128, MC], F32, isOutput=False)
    ubT = nc.declare_dram_parameter("ubT", [128, 256], BF, isOutput=False)
    gtT = nc.declare_dram_parameter("gtT", [128, 512], F32, isOutput=False)
    h0T = nc.declare_dram_parameter("h0T", [128, 256], F32, isOutput=False)
    ident = nc.declare_dram_parameter("ident", [128, 128], BF, isOutput=False)
    out = nc.declare_dram_parameter("out", [128, 256], F32, isOutput=True)

    with tile.TileContext(nc) as tc:
        with (
            tc.tile_pool(name="w", bufs=1) as wp,
            tc.tile_pool(name="ew", bufs=3) as ew,
            tc.tile_pool(name="ps", bufs=1, space="PSUM") as psp,
            tc.tile_pool(name="pp", bufs=1, space="PSUM") as ppp,
        ):
            xT_sb = [wp.tile([128, BT], BF, tag=f"xT{k}", name=f"xT{k}") for k in range(KC)]
            uu_sb = [wp.tile([128, 2048], BF, tag=f"uu{k}", name=f"uu{k}") for k in range(KC)]
            wpre_sb = [wp.tile([128, 2048], BF, tag=f"wp{k}", name=f"wp{k}") for k in range(KC)]
            ring = wp.tile([128, RING * 512], BF, tag="ring")
            bias_sb = wp.tile([128, MC], F32, tag="bias")
            ub_sb = wp.tile([128, 256], BF, tag="ub")
            gt_sb = wp.tile([128, 512], F32, tag="gt")
            id_sb = wp.tile([128, 128], BF, tag="id")
            h0_sb = wp.tile([128, 256], F32, tag="h0")
            hT = wp.tile([128, 256], F32, tag="hT")
            hbf = wp.tile([128, 256], BF, tag="hbf")

            XSL = 4                       # xT DMA column slices
            for k in range(KC):
                nc.sync.dma_start(out=uu_sb[k][:, :], in_=uuT[k * 128:(k + 1) * 128, :])
                nc.sync.dma_start(out=wpre_sb[k][:, :], in_=wpreT[k * 128:(k + 1) * 128, :])
                for s in range(XSL):
                    cs = slice(s * (BT // XSL), (s + 1) * (BT // XSL))
                    nc.sync.dma_start(out=xT_sb[k][:, cs],
                                      in_=xT[k * 128:(k + 1) * 128, cs])
            nc.sync.dma_start(out=gt_sb[:, :], in_=gtT[:, :])
            nc.sync.dma_start(out=bias_sb[:, :], in_=biasp[:, :])
            nc.sync.dma_start(out=ub_sb[:, :], in_=ubT[:, :])
            nc.sync.dma_start(out=id_sb[:, :], in_=ident[:, :])
            nc.sync.dma_start(out=h0_sb[:, :], in_=h0T[:, :])

            ring3 = ring[:, :].rearrange("p (s c) -> p s c", c=512)

            NPRE = 4         # distinct precompute psum buffers

            def precompute_block(i, r):
                # xr/xn for steps i*BLK .. (i+1)*BLK, into ring slots (i%2)*BLK ..
                s0 = (i % 2) * BLK
                for m in range(MC):
                    slot = (i * MC + m) % NPRE
                    ps = ppp.tile([128, BLK * 32], F32, tag=f"pre{slot}",
                                  name=f"pre{r}_{i}_{m}", padded_shape=[128, 512])
                    for k in range(KC):
                        nc.tensor.matmul(
                            ps[:, :],
                            wpre_sb[k][:, m * 128:(m + 1) * 128],
                            xT_sb[k][:, i * BLK * 32:(i + 1) * BLK * 32],
                            start=(k == 0),
                            stop=(k == KC - 1),
                        )
                    dst = ring3[:, s0:s0 + BLK, m * 32:(m + 1) * 32]
                    src = ps[:, :].rearrange("p (s c) -> p s c", c=32)
                    nc.vector.tensor_scalar(dst, src, bias_sb[:, m:m + 1],
                                            None, OP.add)

            def scan_step(t, r):
                base = (t % RING) * 512
                slot = t % 2
                ps_r = psp.tile([128, 256], F32, tag=f"pr{slot}", name=f"psr{r}_{t}",
                                padded_shape=[128, 512])
                ps_n = psp.tile([128, 256], F32, tag=f"pn{slot}", name=f"psn{r}_{t}",
                                padded_shape=[128, 512])
                # Initialize PSUM via identity matmuls:
                # ps_r = xr_t (+r-biases, folded on host into the ring),
                # ps_n = U_b.
                nc.tensor.matmul(ps_r[:, :], id_sb[:, :], ring[:, base:base + 256],
                                 start=True, stop=False, skip_group_check=True)
                nc.tensor.matmul(ps_n[:, :], id_sb[:, :], ub_sb[:, :],
                                 start=True, stop=False, skip_group_check=True)
                # k-half-major: the kh=0 matmuls only read hbf cols 0:128,
                # which the previous step's first elementwise half produced.
                for kh in range(2):
                    for m in range(MC):
                        half = ps_r if m < 8 else ps_n
                        col = (m % 8) * 32
                        for k in range(4 * kh, 4 * kh + 4):
                            nc.tensor.matmul(
                                half[:, col:col + 32],
                                uu_sb[k][:, m * 128:(m + 1) * 128],
                                hbf[:, k * 32:(k + 1) * 32],
                                start=False,
                                stop=(k == KC - 1),
                                skip_group_check=True,
                            )
                for hh in range(2):
                    hs = slice(128 * hh, 128 * (hh + 1))
                    g1 = slice(256 + 128 * hh, 256 + 128 * (hh + 1))
                    xn_b = slice(base + 256 + 128 * hh, base + 256 + 128 * (hh + 1))
                    # g*h term: depends only on the previous step's hT, so it
                    # runs during the matmul burst, off the critical chain.
                    p2 = ew.tile([128, 128], F32, tag=f"p2{hh}", name=f"p2{r}_{t}_{hh}")
                    nc.vector.tensor_tensor(p2[:, :], hT[:, hs], gt_sb[:, hs], OP.mult)
                    rt = ew.tile([128, 128], F32, tag=f"rt{hh}", name=f"rt{r}_{t}_{hh}")
                    nc.scalar.activation(rt[:, :], ps_r[:, hs], AF.Sigmoid)
                    n2 = ew.tile([128, 128], F32, tag=f"n2{hh}", name=f"n2{r}_{t}_{hh}")
                    nc.vector.tensor_tensor(n2[:, :], rt[:, :], ps_n[:, hs], OP.mult)
                    an = ew.tile([128, 128], F32, tag=f"an{hh}", name=f"an{r}_{t}_{hh}")
                    nc.vector.tensor_tensor(an[:, :], n2[:, :], ring[:, xn_b], OP.add)
                    nt = ew.tile([128, 128], F32, tag=f"nt{hh}", name=f"nt{r}_{t}_{hh}")
                    nc.scalar.activation(nt[:, :], an[:, :], AF.Tanh)
                    p1 = ew.tile([128, 128], F32, tag=f"p1{hh}", name=f"p1{r}_{t}_{hh}")
                    nc.vector.tensor_tensor(p1[:, :], nt[:, :], gt_sb[:, g1], OP.mult)
                    # critical: hbf feeds the next step's matmuls
                    nc.vector.tensor_tensor(hbf[:, hs], p1[:, :], p2[:, :], OP.add)
                    # fp32 state copy, off the critical chain
                    nc.gpsimd.tensor_tensor(hT[:, hs], p1[:, :], p2[:, :], OP.add)

            for r in range(reps):
                nc.vector.tensor_copy(hT[:, :], h0_sb[:, :])
                nc.vector.tensor_copy(hbf[:, :], h0_sb[:, :])
                precompute_block(0, r)
                precompute_block(1, r)
                for i in range(NBLK):
                    for u in range(BLK):
                        scan_step(i * BLK + u, r)
                    if i + 2 < NBLK:
                        precompute_block(i + 2, r)

            nc.sync.dma_start(out=out[:, :], in_=hT[:, :])

    nc.finalize()
    return nc


def _prep_inputs(x, h0, gt, Wr_w, Wr_b, Ur_w, Ur_b, W_w, W_b, U_w, U_b):
    bf = ml_dtypes.bfloat16
    wpreT = np.ascontiguousarray(
        np.concatenate([Wr_w.T, W_w.T], axis=1)).astype(bf)          # [H, 2048]
    uuT = np.ascontiguousarray(
        np.concatenate([Ur_w.T, U_w.T], axis=1)).astype(bf)          # [H, 2048]
    # biasp[:, m] = per-partition bias for precompute chunk m, added during
    # the psum->ring copy: r-chunks get Wr_b+Ur_b, n-chunks get W_b.
    # ubT = U_b broadcast (folded into ps_n by an identity matmul).
    def hmajor_bcast(v):
        return np.ascontiguousarray(
            np.broadcast_to(v.reshape(8, 128).T[:, :, None],
                            (128, 8, 32)).reshape(128, 256))
    bias_cat = np.concatenate([(Wr_b + Ur_b), W_b]).astype(np.float32)
    biasp = np.ascontiguousarray(bias_cat.reshape(MC, 128).T)
    ubT = hmajor_bcast(U_b.astype(np.float32)).astype(bf)
    ident = np.eye(128, dtype=bf)

    in_maps = []
    for c in range(NCORES):
        sl = slice(c * BL, (c + 1) * BL)
        x_loc = x[sl]                                  # [32, 128, 1024]
        xT = np.ascontiguousarray(
            x_loc.transpose(2, 1, 0).reshape(H, BT)).astype(bf)
        h0T = np.ascontiguousarray(
            h0[sl].reshape(BL, 8, 128).transpose(2, 1, 0).reshape(128, 256)
        ).astype(np.float32)
        g_b = np.ascontiguousarray(
            np.broadcast_to(gt[sl].reshape(BL)[None, None, :],
                            (128, 8, 32)).reshape(128, 256)).astype(np.float32)
        gtT = np.ascontiguousarray(np.concatenate([g_b, 1.0 - g_b], axis=1))
        in_maps.append({
            "xT": xT, "wpreT": wpreT, "uuT": uuT, "biasp": biasp,
            "ubT": ubT, "gtT": gtT, "h0T": h0T, "ident": ident,
        })
    return in_maps


def kernel(x, h0, gt, Wr_w, Wr_b, Ur_w, Ur_b, Wz_w, Wz_b, Uz_w, Uz_b,
           W_w, W_b, U_w, U_b, _trace=False, _tmpdir=None):
    x = np.asarray(x, np.float32)
    h0 = np.asarray(h0, np.float32)
    gt = np.asarray(gt, np.float32)
    in_maps = _prep_inputs(x, h0, gt,
                           np.asarray(Wr_w, np.float32), np.asarray(Wr_b, np.float32),
                           np.asarray(Ur_w, np.float32), np.asarray(Ur_b, np.float32),
                           np.asarray(W_w, np.float32), np.asarray(W_b, np.float32),
                           np.asarray(U_w, np.float32), np.asarray(U_b, np.float32))
    if "nc" not in _CACHE:
        _CACHE["nc"] = _build_bass()
    res = run_bass_kernel_spmd(_CACHE["nc"], in_maps, core_ids=list(range(NCORES)),
                               trace=_trace, tmpdir=_tmpdir)
    outs = []
    for c in range(NCORES):
        o = np.asarray(res.results[c]["out"], np.float32)       # [128, 256]
        outs.append(o.reshape(128, 8, BL).transpose(2, 1, 0).reshape(BL, H))
    full = np.concatenate(outs, axis=0)                          # [256, 1024]
    if _trace:
        return full, res
    return full
